# revision 8
# baseline (speedup 1.0000x reference)
"""Block-circulant linear (MINI_BLOCK=4) via length-4 rFFT factorization on 8 trn2 cores.

Math: out = x @ W^T where W[4y+n, 4x+j] = eigens[y, x, (n-j) mod 4].
In the length-4 DFT domain the circulant contraction factors into 6 real
matmuls over the block-index axis gx=1024 (10.7x fewer FLOPs than dense):
  X0 = x0+x1+x2+x3, X1 = (x0-x2) + i(x3-x1), X2 = x0-x1+x2-x3  (per block of 4)
  Y0 = X0 E0, Y1 = X1*E1 (complex), Y2 = X2 E2   (contract over gx)
  o0 = Y0+2Re(Y1)+Y2, o1 = Y0-2Im(Y1)-Y2, o2 = Y0-2Re(Y1)+Y2, o3 = Y0+2Im(Y1)-Y2 (/4)

Sharding: data-parallel over batch, 512 rows per core; E-matrices (host
pre-transformed from eigens, scales folded) replicated per core. The x shard
is shipped host-transposed (pure layout) so the contraction axis lands on
SBUF partitions without any on-device transposes; the DFT butterflies are
unit-stride vector adds. Matmuls run in float32r (fp32 bits, reduced-precision
PE multiply, 4x faster than fp32; rel err ~2e-4 over K=1024).
"""
import numpy as np

B, IN, OUT, BLK = 4096, 4096, 4096, 4
GX, GY = IN // BLK, OUT // BLK        # 1024, 1024
NCORES = 8
BS = B // NCORES                      # 512 batch rows per core
BT = BS // 128                        # 4 b-tiles
XC = GX // 128                        # 8 x-chunks (contraction)
YCS = 256                             # y-chunk size (matmul N)
YCN = GY // YCS                       # 2 y-chunks

_cache = {}


def _build_nc():
    from concourse import bacc
    import concourse.mybir as mybir
    from concourse.tile import TileContext

    f32 = mybir.dt.float32
    f32r = mybir.dt.float32r
    bf16 = mybir.dt.bfloat16

    nc = bacc.Bacc("TRN2", target_bir_lowering=False, debug=False,
                   enable_asserts=False, num_devices=NCORES)
    # x shard, transposed on host: [IN, BS] so the block axis is the DMA
    # partition axis.
    xt_d = nc.dram_tensor("xst", [IN, BS], bf16, kind="ExternalInput")
    e_d = [nc.dram_tensor(nm, [YCN, XC, 128, YCS], f32r, kind="ExternalInput")
           for nm in ("e0", "e1r", "e1i", "e2")]
    out_d = nc.dram_tensor("out", [BS, OUT], f32, kind="ExternalOutput")

    with TileContext(nc) as tc:
        with (
            tc.tile_pool(name="xload", bufs=3) as xpool,
            tc.tile_pool(name="xt", bufs=1) as xtp,
            tc.tile_pool(name="epool", bufs=2) as ep,
            tc.tile_pool(name="outp", bufs=3) as op_,
            tc.tile_pool(name="comb", bufs=2) as cb,
            tc.tile_pool(name="mpsum", bufs=1, space="PSUM") as mps,
        ):
            # Forward DFT of x, contraction-major: xt[k] is [x-part, xc, b].
            # yc=0's E chunks are loaded interleaved per-xc with the x loads
            # so the first matmul chain can start after ~1.5 MB of DMA.
            xt = [xtp.tile([128, XC, BS], f32r, tag=f"xt{k}", name=f"xt{k}")
                  for k in range(4)]  # 0 -> X0, 1 -> X1r, 2 -> X1i, 3 -> X2
            et0 = [ep.tile([128, XC, YCS], f32r, tag=f"e{k}", name=f"et{k}")
                   for k in range(4)]
            for xc in range(XC):
                xj = []
                for j in range(4):
                    t = xpool.tile([128, BS], bf16, tag=f"xj{j}", name=f"xj{j}", bufs=4)
                    # rows 4*(128*xc + p) + j of xst, p = 0..127
                    nc.sync.dma_start(
                        out=t,
                        in_=xt_d[:, :].rearrange("(c p j) b -> c j p b", p=128, j=4)[xc, j])
                    xj.append(t)
                for k in range(4):
                    nc.scalar.dma_start(out=et0[k][:, xc], in_=e_d[k][0, xc])
                s02 = xpool.tile([128, BS], f32r, tag="s02")
                s13 = xpool.tile([128, BS], f32r, tag="s13")
                nc.vector.tensor_add(out=s02, in0=xj[0], in1=xj[2])
                nc.vector.tensor_add(out=s13, in0=xj[1], in1=xj[3])
                nc.vector.tensor_sub(out=xt[1][:, xc], in0=xj[0], in1=xj[2])
                nc.vector.tensor_sub(out=xt[2][:, xc], in0=xj[3], in1=xj[1])
                nc.vector.tensor_add(out=xt[0][:, xc], in0=s02, in1=s13)
                nc.vector.tensor_sub(out=xt[3][:, xc], in0=s02, in1=s13)

            # Main: 6 matmul chains per (yc, bt), inverse DFT, store
            for yc in range(YCN):
                if yc == 0:
                    et = et0
                else:
                    et = [ep.tile([128, XC, YCS], f32r, tag=f"e{k}", name=f"et{k}")
                          for k in range(4)]
                    for k in range(4):
                        for xc in range(XC):
                            nc.scalar.dma_start(out=et[k][:, xc], in_=e_d[k][yc, xc])
                for bt in range(BT):
                    bsl = slice(bt * 128, (bt + 1) * 128)
                    y0 = mps.tile([128, YCS], f32, tag="y0")
                    y2 = mps.tile([128, YCS], f32, tag="y2")
                    p_ = mps.tile([128, YCS], f32, tag="p", bufs=2)   # X1r E1r
                    q_ = mps.tile([128, YCS], f32, tag="q", bufs=2)   # X1i E1i
                    yi = mps.tile([128, YCS], f32, tag="yi", bufs=2)  # X1i E1r + X1r E1i
                    # yi (the longest chain, freed mid-combine) runs first so
                    # its bank recycles a full chain-length ahead of reuse.
                    for xc in range(XC):
                        nc.tensor.matmul(yi, xt[2][:, xc, bsl], et[1][:, xc],
                                         start=xc == 0, stop=False)
                    for xc in range(XC):
                        nc.tensor.matmul(yi, xt[1][:, xc, bsl], et[2][:, xc],
                                         start=False, stop=xc == XC - 1)
                    for xc in range(XC):
                        st, sp = xc == 0, xc == XC - 1
                        nc.tensor.matmul(y0, xt[0][:, xc, bsl], et[0][:, xc], start=st, stop=sp)
                        nc.tensor.matmul(y2, xt[3][:, xc, bsl], et[3][:, xc], start=st, stop=sp)
                        nc.tensor.matmul(p_, xt[1][:, xc, bsl], et[1][:, xc], start=st, stop=sp)
                        nc.tensor.matmul(q_, xt[2][:, xc, bsl], et[2][:, xc], start=st, stop=sp)
                    # inverse DFT, ops ordered to free PSUM banks in chain
                    # order; DVE/ACT read at most ONE PSUM operand per op.
                    t_ = cb.tile([128, YCS], f32, tag="t")
                    u_ = cb.tile([128, YCS], f32, tag="u")
                    a_ = cb.tile([128, YCS], f32, tag="a")
                    b_ = cb.tile([128, YCS], f32, tag="b")
                    c_ = cb.tile([128, YCS], f32, tag="c")
                    ot = op_.tile([128, 4 * YCS], f32, tag="ot")
                    ov = ot.rearrange("p (y j) -> p y j", j=4)
                    nc.scalar.copy(out=t_, in_=y0)               # frees y0
                    nc.vector.tensor_sub(out=b_, in0=t_, in1=y2) # Y0-Y2
                    nc.vector.tensor_add(out=a_, in0=y2, in1=t_) # Y0+Y2, frees y2
                    nc.vector.tensor_sub(out=ov[:, :, 1], in0=b_, in1=yi)
                    nc.vector.tensor_add(out=ov[:, :, 3], in0=b_, in1=yi)  # frees yi
                    nc.scalar.mul(u_, q_, -1.0)                  # frees q
                    nc.vector.tensor_add(out=c_, in0=p_, in1=u_) # Y1r = P-Q, frees p
                    nc.vector.tensor_add(out=ov[:, :, 0], in0=a_, in1=c_)
                    nc.vector.tensor_sub(out=ov[:, :, 2], in0=a_, in1=c_)
                    nc.sync.dma_start(
                        out=out_d[bsl, yc * 4 * YCS:(yc + 1) * 4 * YCS], in_=ot)
    nc.compile()
    return nc


def _prep_eigens(eigens):
    """eigens (gy, gx, 4) -> four (YCN, XC, 128, YCS) f32 chunked E-matrices,
    transposed to [x, y] with irfft scale factors folded in."""
    e = np.ascontiguousarray(eigens.transpose(1, 0, 2)).astype(np.float32)  # (x, y, j)
    e0 = ((e[..., 0] + e[..., 2]) + (e[..., 1] + e[..., 3])) * 0.25
    e2 = ((e[..., 0] + e[..., 2]) - (e[..., 1] + e[..., 3])) * 0.25
    e1r = (e[..., 0] - e[..., 2]) * 0.5
    e1i = (e[..., 3] - e[..., 1]) * 0.5

    def chunk(m):  # (GX, GY) -> (YCN, XC, 128, YCS)
        return np.ascontiguousarray(
            m.reshape(XC, 128, YCN, YCS).transpose(2, 0, 1, 3))
    return chunk(e0), chunk(e1r), chunk(e1i), chunk(e2)


def _in_maps(x, eigens):
    import ml_dtypes
    x = np.ascontiguousarray(x, dtype=np.float32)
    e0, e1r, e1i, e2 = _prep_eigens(np.asarray(eigens))
    xT = np.ascontiguousarray(x.T).astype(ml_dtypes.bfloat16)  # [IN, B]
    return [
        {"xst": np.ascontiguousarray(xT[:, c * BS:(c + 1) * BS]),
         "e0": e0, "e1r": e1r, "e1i": e1i, "e2": e2}
        for c in range(NCORES)
    ]


def kernel(x, eigens):
    from concourse.bass_utils import run_bass_kernel_spmd

    if "nc" not in _cache:
        _cache["nc"] = _build_nc()
    res = run_bass_kernel_spmd(_cache["nc"], _in_maps(x, eigens),
                               core_ids=list(range(NCORES)))
    return np.concatenate([r["out"] for r in res.results], axis=0)


# revision 9
# speedup vs baseline: 1.1916x; 1.1916x over previous
"""Block-circulant linear (MINI_BLOCK=4) via length-4 rFFT factorization on 8 trn2 cores.

Math: out = x @ W^T where W[4y+n, 4x+j] = eigens[y, x, (n-j) mod 4].
In the length-4 DFT domain the circulant contraction factors into 6 real
matmuls over the block-index axis gx=1024 (10.7x fewer FLOPs than dense):
  X0 = x0+x1+x2+x3, X1 = (x0-x2) + i(x3-x1), X2 = x0-x1+x2-x3  (per block of 4)
  Y0 = X0 E0, Y1 = X1*E1 (complex), Y2 = X2 E2   (contract over gx)
  o0 = Y0+2Re(Y1)+Y2, o1 = Y0-2Im(Y1)-Y2, o2 = Y0-2Re(Y1)+Y2, o3 = Y0+2Im(Y1)-Y2 (/4)

Sharding: data-parallel over batch, 512 rows per core; E-matrices (host
pre-transformed from eigens, scales folded) replicated per core. The x shard
is shipped host-transposed (pure layout) so the contraction axis lands on
SBUF partitions without any on-device transposes; the DFT butterflies are
unit-stride vector adds. Matmuls run in float32r (fp32 bits, reduced-precision
PE multiply, 4x faster than fp32; rel err ~2e-4 over K=1024).
"""
import numpy as np

B, IN, OUT, BLK = 4096, 4096, 4096, 4
GX, GY = IN // BLK, OUT // BLK        # 1024, 1024
NCORES = 8
BS = B // NCORES                      # 512 batch rows per core
BT = BS // 128                        # 4 b-tiles
XC = GX // 128                        # 8 x-chunks (contraction)
YCS = 256                             # y-chunk size (matmul N)
YCN = GY // YCS                       # 2 y-chunks

_cache = {}


def _build_nc():
    from concourse import bacc
    import concourse.mybir as mybir
    from concourse.tile import TileContext

    f32 = mybir.dt.float32
    f32r = mybir.dt.float32r
    bf16 = mybir.dt.bfloat16

    nc = bacc.Bacc("TRN2", target_bir_lowering=False, debug=False,
                   enable_asserts=False, num_devices=NCORES)
    # x shard, transposed on host: [IN, BS] so the block axis is the DMA
    # partition axis.
    xt_d = nc.dram_tensor("xst", [IN, BS], bf16, kind="ExternalInput")
    e_d = [nc.dram_tensor(nm, [YCN, XC, 128, YCS], f32r, kind="ExternalInput")
           for nm in ("e0", "e1r", "e1i", "e2")]
    out_d = nc.dram_tensor("out", [BS, OUT], f32, kind="ExternalOutput")

    with TileContext(nc) as tc:
        with (
            tc.tile_pool(name="xload", bufs=3) as xpool,
            tc.tile_pool(name="xt", bufs=1) as xtp,
            tc.tile_pool(name="epool", bufs=2) as ep,
            tc.tile_pool(name="outp", bufs=3) as op_,
            tc.tile_pool(name="comb", bufs=2) as cb,
            tc.tile_pool(name="mpsum", bufs=1, space="PSUM") as mps,
        ):
            # Forward DFT of x, contraction-major: xt[k] is [x-part, xc, b].
            # yc=0's E chunks are loaded interleaved per-xc with the x loads
            # so the first matmul chain can start after ~1.5 MB of DMA.
            xt = [xtp.tile([128, XC, BS], f32r, tag=f"xt{k}", name=f"xt{k}")
                  for k in range(4)]  # 0 -> X0, 1 -> X1r, 2 -> X1i, 3 -> X2
            et0 = [ep.tile([128, XC, YCS], f32r, tag=f"e{k}", name=f"et{k}")
                   for k in range(4)]
            for xc in range(XC):
                xj = []
                for j in range(4):
                    t = xpool.tile([128, BS], bf16, tag=f"xj{j}", name=f"xj{j}", bufs=4)
                    # rows 4*(128*xc + p) + j of xst, p = 0..127
                    nc.sync.dma_start(
                        out=t,
                        in_=xt_d[:, :].rearrange("(c p j) b -> c j p b", p=128, j=4)[xc, j])
                    xj.append(t)
                for k in range(4):
                    nc.gpsimd.dma_start(out=et0[k][:, xc], in_=e_d[k][0, xc])
                s02 = xpool.tile([128, BS], f32r, tag="s02")
                s13 = xpool.tile([128, BS], f32r, tag="s13")
                nc.vector.tensor_add(out=s02, in0=xj[0], in1=xj[2])
                nc.vector.tensor_add(out=s13, in0=xj[1], in1=xj[3])
                nc.vector.tensor_sub(out=xt[1][:, xc], in0=xj[0], in1=xj[2])
                nc.vector.tensor_sub(out=xt[2][:, xc], in0=xj[3], in1=xj[1])
                nc.vector.tensor_add(out=xt[0][:, xc], in0=s02, in1=s13)
                nc.vector.tensor_sub(out=xt[3][:, xc], in0=s02, in1=s13)

            # Main: 6 matmul chains per (yc, bt), inverse DFT, store
            for yc in range(YCN):
                if yc == 0:
                    et = et0
                else:
                    et = [ep.tile([128, XC, YCS], f32r, tag=f"e{k}", name=f"et{k}")
                          for k in range(4)]
                    for k in range(4):
                        for xc in range(XC):
                            nc.gpsimd.dma_start(out=et[k][:, xc], in_=e_d[k][yc, xc])
                for bt in range(BT):
                    bsl = slice(bt * 128, (bt + 1) * 128)
                    y0 = mps.tile([128, YCS], f32, tag="y0")
                    y2 = mps.tile([128, YCS], f32, tag="y2", bufs=2)
                    p_ = mps.tile([128, YCS], f32, tag="p", bufs=2)   # X1r E1r
                    q_ = mps.tile([128, YCS], f32, tag="q")           # X1i E1i
                    yi = mps.tile([128, YCS], f32, tag="yi", bufs=2)  # X1i E1r + X1r E1i
                    # Round-robin over PSUM banks: consecutive matmuls into the
                    # same bank serialize fill+drain (~215ns vs ~120ns), so no
                    # two adjacent matmuls may share a target bank.
                    for xc in range(XC):
                        st, sp = xc == 0, xc == XC - 1
                        nc.tensor.matmul(yi, xt[2][:, xc, bsl], et[1][:, xc], start=st, stop=False)
                        nc.tensor.matmul(y0, xt[0][:, xc, bsl], et[0][:, xc], start=st, stop=sp)
                        nc.tensor.matmul(y2, xt[3][:, xc, bsl], et[3][:, xc], start=st, stop=sp)
                        nc.tensor.matmul(yi, xt[1][:, xc, bsl], et[2][:, xc], start=False, stop=sp)
                        nc.tensor.matmul(p_, xt[1][:, xc, bsl], et[1][:, xc], start=st, stop=sp)
                        nc.tensor.matmul(q_, xt[2][:, xc, bsl], et[2][:, xc], start=st, stop=sp)
                    # inverse DFT, ops ordered to free PSUM banks in chain
                    # order; DVE/ACT read at most ONE PSUM operand per op.
                    t_ = cb.tile([128, YCS], f32, tag="t")
                    u_ = cb.tile([128, YCS], f32, tag="u")
                    a_ = cb.tile([128, YCS], f32, tag="a")
                    b_ = cb.tile([128, YCS], f32, tag="b")
                    c_ = cb.tile([128, YCS], f32, tag="c")
                    ot = op_.tile([128, 4 * YCS], f32, tag="ot")
                    ov = ot.rearrange("p (y j) -> p y j", j=4)
                    nc.scalar.copy(out=t_, in_=y0)               # frees y0
                    nc.vector.tensor_sub(out=b_, in0=t_, in1=y2) # Y0-Y2
                    nc.vector.tensor_add(out=a_, in0=y2, in1=t_) # Y0+Y2, frees y2
                    nc.vector.tensor_sub(out=ov[:, :, 1], in0=b_, in1=yi)
                    nc.vector.tensor_add(out=ov[:, :, 3], in0=b_, in1=yi)  # frees yi
                    nc.scalar.mul(u_, q_, -1.0)                  # frees q
                    nc.vector.tensor_add(out=c_, in0=p_, in1=u_) # Y1r = P-Q, frees p
                    nc.vector.tensor_add(out=ov[:, :, 0], in0=a_, in1=c_)
                    nc.vector.tensor_sub(out=ov[:, :, 2], in0=a_, in1=c_)
                    nc.sync.dma_start(
                        out=out_d[bsl, yc * 4 * YCS:(yc + 1) * 4 * YCS], in_=ot)
    nc.compile()
    return nc


def _prep_eigens(eigens):
    """eigens (gy, gx, 4) -> four (YCN, XC, 128, YCS) f32 chunked E-matrices,
    transposed to [x, y] with irfft scale factors folded in."""
    e = np.ascontiguousarray(eigens.transpose(1, 0, 2)).astype(np.float32)  # (x, y, j)
    e0 = ((e[..., 0] + e[..., 2]) + (e[..., 1] + e[..., 3])) * 0.25
    e2 = ((e[..., 0] + e[..., 2]) - (e[..., 1] + e[..., 3])) * 0.25
    e1r = (e[..., 0] - e[..., 2]) * 0.5
    e1i = (e[..., 3] - e[..., 1]) * 0.5

    def chunk(m):  # (GX, GY) -> (YCN, XC, 128, YCS)
        return np.ascontiguousarray(
            m.reshape(XC, 128, YCN, YCS).transpose(2, 0, 1, 3))
    return chunk(e0), chunk(e1r), chunk(e1i), chunk(e2)


def _in_maps(x, eigens):
    import ml_dtypes
    x = np.ascontiguousarray(x, dtype=np.float32)
    e0, e1r, e1i, e2 = _prep_eigens(np.asarray(eigens))
    xT = np.ascontiguousarray(x.T).astype(ml_dtypes.bfloat16)  # [IN, B]
    return [
        {"xst": np.ascontiguousarray(xT[:, c * BS:(c + 1) * BS]),
         "e0": e0, "e1r": e1r, "e1i": e1i, "e2": e2}
        for c in range(NCORES)
    ]


def kernel(x, eigens):
    from concourse.bass_utils import run_bass_kernel_spmd

    if "nc" not in _cache:
        _cache["nc"] = _build_nc()
    res = run_bass_kernel_spmd(_cache["nc"], _in_maps(x, eigens),
                               core_ids=list(range(NCORES)))
    return np.concatenate([r["out"] for r in res.results], axis=0)


# revision 10
# speedup vs baseline: 1.3017x; 1.0924x over previous
"""Block-circulant linear (MINI_BLOCK=4) via length-4 rFFT factorization on 8 trn2 cores.

Math: out = x @ W^T where W[4y+n, 4x+j] = eigens[y, x, (n-j) mod 4].
In the length-4 DFT domain the circulant contraction factors into 6 real
matmuls over the block-index axis gx=1024 (10.7x fewer FLOPs than dense):
  X0 = x0+x1+x2+x3, X1 = (x0-x2) + i(x3-x1), X2 = x0-x1+x2-x3  (per block of 4)
  Y0 = X0 E0, Y1 = X1*E1 (complex), Y2 = X2 E2   (contract over gx)
  o0 = Y0+2Re(Y1)+Y2, o1 = Y0-2Im(Y1)-Y2, o2 = Y0-2Re(Y1)+Y2, o3 = Y0+2Im(Y1)-Y2 (/4)

Sharding: data-parallel over batch, 512 rows per core; E-matrices (host
pre-transformed from eigens, scales folded) replicated per core. The x shard
is shipped host-transposed (pure layout) so the contraction axis lands on
SBUF partitions without any on-device transposes; the DFT butterflies are
unit-stride vector adds. Matmuls run in float32r (fp32 bits, reduced-precision
PE multiply); operands are shipped/stored bf16 (rel err ~3e-3 over K=1024,
fp32 PSUM accumulation), which halves DMA traffic and doubles matmul N.
"""
import numpy as np

B, IN, OUT, BLK = 4096, 4096, 4096, 4
GX, GY = IN // BLK, OUT // BLK        # 1024, 1024
NCORES = 8
BS = B // NCORES                      # 512 batch rows per core
BT = BS // 128                        # 4 b-tiles
XC = GX // 128                        # 8 x-chunks (contraction)
YCS = 512                             # y-chunk size (matmul N)
YCN = GY // YCS                       # 2 y-chunks

_cache = {}


def _build_nc():
    from concourse import bacc
    import concourse.mybir as mybir
    from concourse.tile import TileContext

    f32 = mybir.dt.float32
    f32r = mybir.dt.float32r
    bf16 = mybir.dt.bfloat16

    nc = bacc.Bacc("TRN2", target_bir_lowering=False, debug=False,
                   enable_asserts=False, num_devices=NCORES)
    # x shard, transposed on host: [IN, BS] so the block axis is the DMA
    # partition axis.
    xt_d = nc.dram_tensor("xst", [IN, BS], bf16, kind="ExternalInput")
    e_d = [nc.dram_tensor(nm, [YCN, XC, 128, YCS], bf16, kind="ExternalInput")
           for nm in ("e0", "e1r", "e1i", "e2")]
    out_d = nc.dram_tensor("out", [BS, OUT], f32, kind="ExternalOutput")

    with TileContext(nc) as tc:
        with (
            tc.tile_pool(name="xload", bufs=3) as xpool,
            tc.tile_pool(name="xt", bufs=1) as xtp,
            tc.tile_pool(name="epool", bufs=2) as ep,
            tc.tile_pool(name="outp", bufs=3) as op_,
            tc.tile_pool(name="comb", bufs=2) as cb,
            tc.tile_pool(name="mpsum", bufs=1, space="PSUM") as mps,
        ):
            # Forward DFT of x, contraction-major: xt[k] is [x-part, xc, b].
            # yc=0's E chunks are loaded interleaved per-xc with the x loads
            # so the first matmul chain can start after ~1.5 MB of DMA.
            xt = [xtp.tile([128, XC, BS], bf16, tag=f"xt{k}", name=f"xt{k}")
                  for k in range(4)]  # 0 -> X0, 1 -> X1r, 2 -> X1i, 3 -> X2
            et0 = [ep.tile([128, XC, YCS], bf16, tag=f"e{k}", name=f"et{k}")
                   for k in range(4)]
            for xc in range(XC):
                xj = []
                for j in range(4):
                    t = xpool.tile([128, BS], bf16, tag=f"xj{j}", name=f"xj{j}", bufs=4)
                    # rows 4*(128*xc + p) + j of xst, p = 0..127
                    nc.sync.dma_start(
                        out=t,
                        in_=xt_d[:, :].rearrange("(c p j) b -> c j p b", p=128, j=4)[xc, j])
                    xj.append(t)
                for k in range(4):
                    nc.gpsimd.dma_start(out=et0[k][:, xc], in_=e_d[k][0, xc])
                s02 = xpool.tile([128, BS], f32, tag="s02")
                s13 = xpool.tile([128, BS], f32, tag="s13")
                nc.vector.tensor_add(out=s02, in0=xj[0], in1=xj[2])
                nc.vector.tensor_add(out=s13, in0=xj[1], in1=xj[3])
                nc.vector.tensor_sub(out=xt[1][:, xc], in0=xj[0], in1=xj[2])
                nc.vector.tensor_sub(out=xt[2][:, xc], in0=xj[3], in1=xj[1])
                nc.vector.tensor_add(out=xt[0][:, xc], in0=s02, in1=s13)
                nc.vector.tensor_sub(out=xt[3][:, xc], in0=s02, in1=s13)

            # Main: 6 matmul chains per (yc, bt), inverse DFT, store
            for yc in range(YCN):
                if yc == 0:
                    et = et0
                else:
                    et = [ep.tile([128, XC, YCS], bf16, tag=f"e{k}", name=f"et{k}")
                          for k in range(4)]
                    for k in range(4):
                        for xc in range(XC):
                            nc.gpsimd.dma_start(out=et[k][:, xc], in_=e_d[k][yc, xc])
                for bt in range(BT):
                    bsl = slice(bt * 128, (bt + 1) * 128)
                    y0 = mps.tile([128, YCS], f32, tag="y0")
                    y2 = mps.tile([128, YCS], f32, tag="y2", bufs=2)
                    p_ = mps.tile([128, YCS], f32, tag="p", bufs=2)   # X1r E1r
                    q_ = mps.tile([128, YCS], f32, tag="q")           # X1i E1i
                    yi = mps.tile([128, YCS], f32, tag="yi", bufs=2)  # X1i E1r + X1r E1i
                    # Round-robin over PSUM banks: consecutive matmuls into the
                    # same bank serialize fill+drain (~215ns vs ~120ns), so no
                    # two adjacent matmuls may share a target bank.
                    for xc in range(XC):
                        st, sp = xc == 0, xc == XC - 1
                        nc.tensor.matmul(yi, xt[2][:, xc, bsl], et[1][:, xc], start=st, stop=False)
                        nc.tensor.matmul(y0, xt[0][:, xc, bsl], et[0][:, xc], start=st, stop=sp)
                        nc.tensor.matmul(y2, xt[3][:, xc, bsl], et[3][:, xc], start=st, stop=sp)
                        nc.tensor.matmul(yi, xt[1][:, xc, bsl], et[2][:, xc], start=False, stop=sp)
                        nc.tensor.matmul(p_, xt[1][:, xc, bsl], et[1][:, xc], start=st, stop=sp)
                        nc.tensor.matmul(q_, xt[2][:, xc, bsl], et[2][:, xc], start=st, stop=sp)
                    # inverse DFT, ops ordered to free PSUM banks in chain
                    # order; DVE/ACT read at most ONE PSUM operand per op.
                    t_ = cb.tile([128, YCS], f32, tag="t")
                    u_ = cb.tile([128, YCS], f32, tag="u")
                    a_ = cb.tile([128, YCS], f32, tag="a")
                    b_ = cb.tile([128, YCS], f32, tag="b")
                    c_ = cb.tile([128, YCS], f32, tag="c")
                    ot = op_.tile([128, 4 * YCS], f32, tag="ot")
                    ov = ot.rearrange("p (y j) -> p y j", j=4)
                    nc.scalar.copy(out=t_, in_=y0)               # frees y0
                    nc.vector.tensor_sub(out=b_, in0=t_, in1=y2) # Y0-Y2
                    nc.vector.tensor_add(out=a_, in0=y2, in1=t_) # Y0+Y2, frees y2
                    nc.vector.tensor_sub(out=ov[:, :, 1], in0=b_, in1=yi)
                    nc.vector.tensor_add(out=ov[:, :, 3], in0=b_, in1=yi)  # frees yi
                    nc.scalar.mul(u_, q_, -1.0)                  # frees q
                    nc.vector.tensor_add(out=c_, in0=p_, in1=u_) # Y1r = P-Q, frees p
                    nc.vector.tensor_add(out=ov[:, :, 0], in0=a_, in1=c_)
                    nc.vector.tensor_sub(out=ov[:, :, 2], in0=a_, in1=c_)
                    nc.sync.dma_start(
                        out=out_d[bsl, yc * 4 * YCS:(yc + 1) * 4 * YCS], in_=ot)
    nc.compile()
    return nc


def _prep_eigens(eigens):
    """eigens (gy, gx, 4) -> four (YCN, XC, 128, YCS) f32 chunked E-matrices,
    transposed to [x, y] with irfft scale factors folded in."""
    e = np.ascontiguousarray(eigens.transpose(1, 0, 2)).astype(np.float32)  # (x, y, j)
    e0 = ((e[..., 0] + e[..., 2]) + (e[..., 1] + e[..., 3])) * 0.25
    e2 = ((e[..., 0] + e[..., 2]) - (e[..., 1] + e[..., 3])) * 0.25
    e1r = (e[..., 0] - e[..., 2]) * 0.5
    e1i = (e[..., 3] - e[..., 1]) * 0.5

    import ml_dtypes

    def chunk(m):  # (GX, GY) -> (YCN, XC, 128, YCS)
        return np.ascontiguousarray(
            m.reshape(XC, 128, YCN, YCS).transpose(2, 0, 1, 3)).astype(ml_dtypes.bfloat16)
    return chunk(e0), chunk(e1r), chunk(e1i), chunk(e2)


def _in_maps(x, eigens):
    import ml_dtypes
    x = np.ascontiguousarray(x, dtype=np.float32)
    e0, e1r, e1i, e2 = _prep_eigens(np.asarray(eigens))
    xT = np.ascontiguousarray(x.T).astype(ml_dtypes.bfloat16)  # [IN, B]
    return [
        {"xst": np.ascontiguousarray(xT[:, c * BS:(c + 1) * BS]),
         "e0": e0, "e1r": e1r, "e1i": e1i, "e2": e2}
        for c in range(NCORES)
    ]


def kernel(x, eigens):
    from concourse.bass_utils import run_bass_kernel_spmd

    if "nc" not in _cache:
        _cache["nc"] = _build_nc()
    res = run_bass_kernel_spmd(_cache["nc"], _in_maps(x, eigens),
                               core_ids=list(range(NCORES)))
    return np.concatenate([r["out"] for r in res.results], axis=0)


# revision 12
# speedup vs baseline: 1.3493x; 1.0366x over previous
"""Block-circulant linear (MINI_BLOCK=4) via length-4 rFFT factorization on 8 trn2 cores.

Math: out = x @ W^T where W[4y+n, 4x+j] = eigens[y, x, (n-j) mod 4].
In the length-4 DFT domain the circulant contraction factors into 6 real
matmuls over the block-index axis gx=1024 (10.7x fewer FLOPs than dense):
  X0 = x0+x1+x2+x3, X1 = (x0-x2) + i(x3-x1), X2 = x0-x1+x2-x3  (per block of 4)
  Y0 = X0 E0, Y1 = X1*E1 (complex), Y2 = X2 E2   (contract over gx)
  o0 = Y0+2Re(Y1)+Y2, o1 = Y0-2Im(Y1)-Y2, o2 = Y0-2Re(Y1)+Y2, o3 = Y0+2Im(Y1)-Y2 (/4)

Sharding: data-parallel over batch, 512 rows per core; E-matrices (host
pre-transformed from eigens, scales folded) replicated per core. The x shard
is shipped host-transposed (pure layout) so the contraction axis lands on
SBUF partitions without any on-device transposes; the DFT butterflies are
unit-stride vector adds. Matmuls run in float32r (fp32 bits, reduced-precision
PE multiply); operands are shipped/stored bf16 (rel err ~3e-3 over K=1024,
fp32 PSUM accumulation), which halves DMA traffic and doubles matmul N.
"""
import numpy as np

B, IN, OUT, BLK = 4096, 4096, 4096, 4
GX, GY = IN // BLK, OUT // BLK        # 1024, 1024
NCORES = 8
BS = B // NCORES                      # 512 batch rows per core
BT = BS // 128                        # 4 b-tiles
XC = GX // 128                        # 8 x-chunks (contraction)
YCS = 512                             # y-chunk size (matmul N)
YCN = GY // YCS                       # 2 y-chunks

_cache = {}


def _build_nc():
    from concourse import bacc
    import concourse.mybir as mybir
    from concourse.tile import TileContext

    f32 = mybir.dt.float32
    f32r = mybir.dt.float32r
    bf16 = mybir.dt.bfloat16

    nc = bacc.Bacc("TRN2", target_bir_lowering=False, debug=False,
                   enable_asserts=False, num_devices=NCORES)
    # x shard, transposed on host: [IN, BS] so the block axis is the DMA
    # partition axis.
    xt_d = nc.dram_tensor("xst", [IN, BS], bf16, kind="ExternalInput")
    e_d = [nc.dram_tensor(nm, [YCN, XC, 128, YCS], bf16, kind="ExternalInput")
           for nm in ("e0", "e1r", "ed", "e2", "es")]
    out_d = nc.dram_tensor("out", [BS, OUT], f32, kind="ExternalOutput")

    with TileContext(nc) as tc:
        with (
            tc.tile_pool(name="xload", bufs=3) as xpool,
            tc.tile_pool(name="xt", bufs=1) as xtp,
            tc.tile_pool(name="epool", bufs=2) as ep,
            tc.tile_pool(name="outp", bufs=3) as op_,
            tc.tile_pool(name="comb", bufs=2) as cb,
            tc.tile_pool(name="mpsum", bufs=1, space="PSUM") as mps,
        ):
            # Forward DFT of x, contraction-major: xt[k] is [x-part, xc, b].
            # yc=0's E chunks are loaded interleaved per-xc with the x loads
            # so the first matmul chain can start after ~1.5 MB of DMA.
            xt = [xtp.tile([128, XC, BS], bf16, tag=f"xt{k}", name=f"xt{k}")
                  for k in range(5)]  # X0, X1r, X1i, X2, X1s=X1r+X1i
            et0 = [ep.tile([128, XC, YCS], bf16, tag=f"e{k}", name=f"et{k}")
                   for k in range(5)]  # E0, E1r, Ed=E1i-E1r, E2, Es=E1r+E1i
            for xc in range(XC):
                # feed E on the GpSimd (SWDGE) and Scalar (2nd HWDGE) rings,
                # x on the Sync ring -> three DMA streams in parallel
                for k in (0, 1, 2):
                    nc.gpsimd.dma_start(out=et0[k][:, xc], in_=e_d[k][0, xc])
                for k in (3, 4):
                    nc.scalar.dma_start(out=et0[k][:, xc], in_=e_d[k][0, xc])
                xj = []
                for j in range(4):
                    t = xpool.tile([128, BS], bf16, tag=f"xj{j}", name=f"xj{j}", bufs=4)
                    # rows 4*(128*xc + p) + j of xst, p = 0..127
                    nc.sync.dma_start(
                        out=t,
                        in_=xt_d[:, :].rearrange("(c p j) b -> c j p b", p=128, j=4)[xc, j])
                    xj.append(t)
                s02 = xpool.tile([128, BS], f32, tag="s02")
                s13 = xpool.tile([128, BS], f32, tag="s13")
                nc.vector.tensor_add(out=s02, in0=xj[0], in1=xj[2])
                nc.vector.tensor_add(out=s13, in0=xj[1], in1=xj[3])
                nc.vector.tensor_sub(out=xt[1][:, xc], in0=xj[0], in1=xj[2])
                nc.vector.tensor_sub(out=xt[2][:, xc], in0=xj[3], in1=xj[1])
                nc.vector.tensor_add(out=xt[0][:, xc], in0=s02, in1=s13)
                nc.vector.tensor_sub(out=xt[3][:, xc], in0=s02, in1=s13)
                nc.vector.tensor_add(out=xt[4][:, xc], in0=xt[1][:, xc], in1=xt[2][:, xc])

            # Main: 6 matmul chains per (yc, bt), inverse DFT, store
            for yc in range(YCN):
                if yc == 0:
                    et = et0
                else:
                    et = [ep.tile([128, XC, YCS], bf16, tag=f"e{k}", name=f"et{k}")
                          for k in range(5)]
                    for k in range(5):
                        for xc in range(XC):
                            nc.gpsimd.dma_start(out=et[k][:, xc], in_=e_d[k][yc, xc])
                for bt in range(BT):
                    bsl = slice(bt * 128, (bt + 1) * 128)
                    # Gauss 3-mult for the complex bin:
                    #   g1 = X1s E1r, g2 = X1r Ed, g3 = X1i Es
                    #   Y1r = g1 - g3, Y1i = g1 + g2
                    y0 = mps.tile([128, YCS], f32, tag="y0")
                    y2 = mps.tile([128, YCS], f32, tag="y2", bufs=2)
                    g1 = mps.tile([128, YCS], f32, tag="g1")
                    g2 = mps.tile([128, YCS], f32, tag="g2", bufs=2)
                    g3 = mps.tile([128, YCS], f32, tag="g3", bufs=2)
                    # Round-robin over PSUM banks: consecutive matmuls into the
                    # same bank serialize fill+drain, so no two adjacent
                    # matmuls may share a target bank.
                    for xc in range(XC):
                        st, sp = xc == 0, xc == XC - 1
                        nc.tensor.matmul(g1, xt[4][:, xc, bsl], et[1][:, xc], start=st, stop=sp)
                        nc.tensor.matmul(y0, xt[0][:, xc, bsl], et[0][:, xc], start=st, stop=sp)
                        nc.tensor.matmul(g2, xt[1][:, xc, bsl], et[2][:, xc], start=st, stop=sp)
                        nc.tensor.matmul(y2, xt[3][:, xc, bsl], et[3][:, xc], start=st, stop=sp)
                        nc.tensor.matmul(g3, xt[2][:, xc, bsl], et[4][:, xc], start=st, stop=sp)
                    # inverse DFT, ops ordered to free PSUM banks in chain
                    # order; DVE/ACT read at most ONE PSUM operand per op.
                    t_ = cb.tile([128, YCS], f32, tag="t")
                    v_ = cb.tile([128, YCS], f32, tag="v")
                    a_ = cb.tile([128, YCS], f32, tag="a")
                    b_ = cb.tile([128, YCS], f32, tag="b")
                    c_ = cb.tile([128, YCS], f32, tag="c")
                    d_ = cb.tile([128, YCS], f32, tag="d")
                    ot = op_.tile([128, 4 * YCS], f32, tag="ot")
                    ov = ot.rearrange("p (y j) -> p y j", j=4)
                    nc.scalar.copy(out=t_, in_=y0)               # frees y0
                    nc.vector.tensor_sub(out=b_, in0=t_, in1=y2) # Y0-Y2
                    nc.vector.tensor_add(out=a_, in0=y2, in1=t_) # Y0+Y2, frees y2
                    nc.scalar.copy(out=v_, in_=g1)               # frees g1
                    nc.vector.tensor_sub(out=c_, in0=v_, in1=g3) # Y1r, frees g3
                    nc.vector.tensor_add(out=d_, in0=v_, in1=g2) # Y1i, frees g2
                    nc.vector.tensor_add(out=ov[:, :, 0], in0=a_, in1=c_)
                    nc.vector.tensor_sub(out=ov[:, :, 2], in0=a_, in1=c_)
                    nc.vector.tensor_sub(out=ov[:, :, 1], in0=b_, in1=d_)
                    nc.vector.tensor_add(out=ov[:, :, 3], in0=b_, in1=d_)
                    nc.sync.dma_start(
                        out=out_d[bsl, yc * 4 * YCS:(yc + 1) * 4 * YCS], in_=ot)
    nc.compile()
    return nc


def _prep_eigens(eigens):
    """eigens (gy, gx, 4) -> four (YCN, XC, 128, YCS) f32 chunked E-matrices,
    transposed to [x, y] with irfft scale factors folded in."""
    e = np.ascontiguousarray(eigens.transpose(1, 0, 2)).astype(np.float32)  # (x, y, j)
    e0 = ((e[..., 0] + e[..., 2]) + (e[..., 1] + e[..., 3])) * 0.25
    e2 = ((e[..., 0] + e[..., 2]) - (e[..., 1] + e[..., 3])) * 0.25
    e1r = (e[..., 0] - e[..., 2]) * 0.5
    e1i = (e[..., 3] - e[..., 1]) * 0.5

    import ml_dtypes

    def chunk(m):  # (GX, GY) -> (YCN, XC, 128, YCS)
        return np.ascontiguousarray(
            m.reshape(XC, 128, YCN, YCS).transpose(2, 0, 1, 3)).astype(ml_dtypes.bfloat16)
    return (chunk(e0), chunk(e1r), chunk(e1i - e1r), chunk(e2),
            chunk(e1r + e1i))


def _in_maps(x, eigens):
    import ml_dtypes
    x = np.ascontiguousarray(x, dtype=np.float32)
    e0, e1r, ed, e2, es = _prep_eigens(np.asarray(eigens))
    xT = np.ascontiguousarray(x.T).astype(ml_dtypes.bfloat16)  # [IN, B]
    return [
        {"xst": np.ascontiguousarray(xT[:, c * BS:(c + 1) * BS]),
         "e0": e0, "e1r": e1r, "ed": ed, "e2": e2, "es": es}
        for c in range(NCORES)
    ]


def kernel(x, eigens):
    from concourse.bass_utils import run_bass_kernel_spmd

    if "nc" not in _cache:
        _cache["nc"] = _build_nc()
    res = run_bass_kernel_spmd(_cache["nc"], _in_maps(x, eigens),
                               core_ids=list(range(NCORES)))
    return np.concatenate([r["out"] for r in res.results], axis=0)
